# revision 36
# baseline (speedup 1.0000x reference)
"""DiT block kernel for 8 Trainium2 NeuronCores (self-contained).

Sharding: sequence-parallel over padded S (3600 -> 4096, 512 rows/core) for
LN/modulate/qkvo/attention/cross-attn; hidden-dim tensor-parallel FFN
(8960 -> 1120/core, padded 1152). Collectives: AllGather(v), AllGather(kT)
for self-attention, AllGather(hT) + ReduceScatter(y2T) for the FFN.
Matmuls in bf16 (fp32 accumulate), residual spine fp32.
"""

import numpy as np
import ml_dtypes

import concourse.bacc as bacc
import concourse.bass as bass
import concourse.mybir as mybir
import concourse.tile as tile
from concourse.masks import make_identity
from concourse.bass_utils import run_bass_kernel_spmd

F32 = mybir.dt.float32
BF16 = mybir.dt.bfloat16
AF = mybir.ActivationFunctionType
ALU = mybir.AluOpType

N_CORES = 8
S = 3600
SP = 4096            # padded sequence
SH = 512             # rows per core
D = 1536
H = 12
HD = 128
LC = 512             # context length
FFN = 8960
FSH = 1120           # ffn hidden per core
FSHP = 1152          # padded (9 * 128)
NKT = 29             # kpos tiles covering rows 0..3712 (>=3600)
EPS = 1e-6
SCALE = float(HD) ** -0.5
NEG = -80.0          # additive mask for padded k positions

BF = ml_dtypes.bfloat16
SIM_MODE = False     # replace collectives with local DMAs (for TimelineSim)


def build():
    nc = bacc.Bacc(num_devices=N_CORES)

    # ---------------- I/O ----------------
    io = {}
    io["x_sh"] = nc.dram_tensor("x_sh", [SH, D], F32, kind="ExternalInput")
    io["ctx_bf"] = nc.dram_tensor("ctx_bf", [LC, D], BF16, kind="ExternalInput")
    io["modul"] = nc.dram_tensor("modul", [6, D], F32, kind="ExternalInput")
    io["t_mod"] = nc.dram_tensor("t_mod", [6, D], F32, kind="ExternalInput")
    io["cos_dup"] = nc.dram_tensor("cos_dup", [128, SH], BF16, kind="ExternalInput")
    io["sin_dup"] = nc.dram_tensor("sin_dup", [128, SH], BF16, kind="ExternalInput")
    io["kmask"] = nc.dram_tensor("kmask", [128, 1], F32, kind="ExternalInput")

    wname = dict(
        sa_qw_p=[H, 128, D], sa_kw_p=[H, 128, D], sa_vw=[H, 128, D], sa_ow=[H, 128, D],
        ca_qw=[H, 128, D], ca_kw=[H, 128, D], ca_vw=[H, 128, D], ca_ow=[H, 128, D],
        w1_s=[128, 9 * H * 128], w2_s=[128, H * 9 * 128],
    )
    W = {k: nc.dram_tensor(k, v, BF16, kind="ExternalInput") for k, v in wname.items()}

    cname = [
        "sa_qb_c", "sa_kb_c", "sa_nq_c", "sa_nk_c", "sa_vb_c",
        "ca_qb_c", "ca_kb_c", "ca_nq_c", "ca_nk_c", "ca_vb_c",
    ]
    C = {k: nc.dram_tensor(k, [128, H], F32, kind="ExternalInput") for k in cname}
    C["b1_c"] = nc.dram_tensor("b1_c", [128, 9], F32, kind="ExternalInput")

    rname = ["n3w_r", "n3b_r", "b2_r", "sa_ob_r", "ca_ob_r"]
    Rr = {k: nc.dram_tensor(k, [1, D], F32, kind="ExternalInput") for k in rname}

    y_out = nc.dram_tensor("y_out", [SH, D], F32, kind="ExternalOutput")

    # internal DRAM
    cc_k_in = nc.dram_tensor("cc_k_in", [D, SH], BF16)
    cc_k_out = nc.dram_tensor("cc_k_out", [N_CORES, D, SH], BF16, addr_space="Shared")
    cc_v_in = nc.dram_tensor("cc_v_in", [4, H, 128, 128], BF16)
    cc_v_out = nc.dram_tensor("cc_v_out", [N_CORES, 4, H, 128, 128], BF16, addr_space="Shared")
    cc_h_in = nc.dram_tensor("cc_h_in", [D, SH], BF16)
    cc_h_out = nc.dram_tensor("cc_h_out", [N_CORES, D, SH], BF16, addr_space="Shared")
    cc_y_in = nc.dram_tensor("cc_y_in", [N_CORES, D, SH], BF16)
    cc_y_out = nc.dram_tensor("cc_y_out", [D, SH], BF16)
    cc_y_in2 = nc.dram_tensor("cc_y_in2", [N_CORES, D, SH], BF16)
    cc_y_out2 = nc.dram_tensor("cc_y_out2", [D, SH], BF16)
    scr_rows = nc.dram_tensor("scr_rows", [10, D], F32)
    ca_k_dram = nc.dram_tensor("ca_k_dram", [D, LC], BF16)
    ca_v_dram = nc.dram_tensor("ca_v_dram", [LC, D], BF16)
    RG = [list(range(N_CORES))]

    dram = dict(scr_rows=scr_rows, cc_k_in=cc_k_in, cc_k_out=cc_k_out, cc_v_in=cc_v_in,
                cc_v_out=cc_v_out, cc_h_in=cc_h_in, cc_h_out=cc_h_out,
                cc_y_in=cc_y_in, cc_y_out=cc_y_out, cc_y_in2=cc_y_in2,
                cc_y_out2=cc_y_out2, ca_k_dram=ca_k_dram,
                ca_v_dram=ca_v_dram)

    with tile.TileContext(nc) as tc:
        _body(nc, tc, io, W, C, Rr, y_out, dram, RG)

    nc.compile()
    return nc


def _body(nc, tc, io, W, C, Rr, y_out, dram, RG):
    ctx = {}

    with (
        tc.tile_pool(name="G", bufs=1) as G,
        tc.tile_pool(name="PS", bufs=2, space="PSUM") as PS_,
        tc.tile_pool(name="PS3", bufs=3, space="PSUM") as PS3,
    ):
        class _PSMux:
            def tile(self, shape, dtype, tag="mm", name=None):
                if tag == "mm":
                    return PS3.tile(shape, dtype, tag="mm", name=name or "psmm")
                if tag == "den":
                    return PS_.tile(shape, dtype, tag="tr", name=name or "psden")
                return PS_.tile(shape, dtype, tag=tag, name=name or "pst")
        PS = _PSMux()
        # ----- x first (so LN starts ASAP) -----
        x_acc = G.tile([128, 4, D], F32)
        xr = io["x_sh"][:, :].rearrange("(rt p) c -> rt p c", p=128)
        for rt in range(4):
            nc.sync.dma_start(out=x_acc[:, rt, :], in_=xr[rt])

        # ----- global constants -----
        ident_bf = G.tile([128, 128], BF16)
        make_identity(nc, ident_bf)
        ident_f = G.tile([128, 128], F32)
        make_identity(nc, ident_f)
        ones_bf = G.tile([128, 1], BF16)
        nc.vector.memset(ones_bf, 1.0)
        ones_f = G.tile([128, 1], F32)
        nc.vector.memset(ones_f, 1.0)
        eps_t = G.tile([128, 1], F32)
        nc.vector.memset(eps_t, EPS)
        kmask_t = G.tile([128, 1], F32)
        nc.sync.dma_start(out=kmask_t, in_=io["kmask"][:, :])

        BC = {}
        for k, t in C.items():
            BC[k] = G.tile(list(t.shape), F32, tag="bc_" + k, name="bct_" + k)
            nc.sync.dma_start(out=BC[k], in_=t[:, :])

        def rowbc_ap(a, n=128):
            return bass.AP(tensor=a.tensor, offset=a.offset, ap=[[0, n], [1, D]])

        def colv_ap(a):
            return bass.AP(tensor=a.tensor, offset=a.offset, ap=[[1, 128], [128, H]])

        # shared staging
        h_bf = G.tile([128, 4, D], BF16)
        hT = G.tile([128, H, SH], BF16)

        def ln_stats(pool, xt):
            stats = pool.tile([128, 3, 6], F32, tag="ln_st", name="ln_st")
            xg = xt.rearrange("p (g f) -> p g f", g=3)
            for g in range(3):
                nc.vector.bn_stats(out=stats[:, g, :], in_=xg[:, g, :])
            mv = pool.tile([128, 2], F32, tag="ln_mv", name="ln_mv")
            nc.vector.bn_aggr(out=mv, in_=stats)
            rstd = pool.tile([128, 1], F32, tag="ln_rs", name="ln_rs")
            nc.scalar.activation(out=rstd, in_=mv[:, 1:2], func=AF.Sqrt, bias=eps_t, scale=1.0)
            nc.vector.reciprocal(out=rstd, in_=rstd)
            return mv, rstd

        def ln_rows(pool, rt):
            """raw LN(x_acc[:,rt,:]) -> h_bf[:,rt,:] (affine applied in transpose drain)."""
            xt = x_acc[:, rt, :]
            mv, rstd = ln_stats(pool, xt)
            nc.vector.tensor_scalar(out=h_bf[:, rt, :], in0=xt, scalar1=mv[:, 0:1],
                                    scalar2=rstd, op0=ALU.subtract, op1=ALU.mult)

        def rows_to_T1(dst_T, src_rows, rt, idn, dt_cast, sc_c=None, sh_c=None):
            for dt0 in range(0, H, 4):
                pst = PS.tile([128, 4, 128], dt_cast, tag="tr", name="pst")
                for j in range(4):
                    nc.tensor.transpose(pst[:, j, :],
                                        src_rows[:, rt, (dt0 + j) * 128:(dt0 + j + 1) * 128], idn)
                if sc_c is None:
                    nc.scalar.copy(
                        out=dst_T[:, dt0:dt0 + 4, rt * 128:(rt + 1) * 128], in_=pst)
                else:
                    for j in range(4):
                        nc.scalar.activation(
                            out=dst_T[:, dt0 + j, rt * 128:(rt + 1) * 128], in_=pst[:, j, :],
                            func=AF.Identity, bias=sh_c[:, dt0 + j:dt0 + j + 1],
                            scale=sc_c[:, dt0 + j:dt0 + j + 1])

        def rows_to_T(dst_T, src_rows, idn, dt_cast):
            """[128,4,D] rows -> dst_T [128, 12, 512] via PE transposes."""
            for rt in range(4):
                for dt0 in range(0, H, 4):
                    pst = PS.tile([128, 4, 128], dt_cast, tag="tr", name="pst")
                    for j in range(4):
                        nc.tensor.transpose(pst[:, j, :],
                                            src_rows[:, rt, (dt0 + j) * 128:(dt0 + j + 1) * 128], idn)
                    nc.vector.tensor_copy(
                        out=dst_T[:, dt0:dt0 + 4, rt * 128:(rt + 1) * 128], in_=pst)

        def T_to_rows(pool, src_T, rt, idn, dt_cast, tag):
            """transpose src_T[:, :, rt-block] back to a [128, D] row tile."""
            orow = pool.tile([128, D], dt_cast, tag="ln_h2" if tag == "orow" else tag, name=tag)
            for dt0 in range(0, H, 4):
                pst = PS.tile([128, 4, 128], dt_cast, tag="tr", name="pst2")
                for j in range(4):
                    nc.tensor.transpose(pst[:, j, :],
                                        src_T[:, dt0 + j, rt * 128:(rt + 1) * 128], idn)
                nc.vector.tensor_copy(out=orow[:, dt0 * 128:(dt0 + 4) * 128], in_=pst)
            return orow

        def wtiles(pool, w_dram, mt):
            t = pool.tile([128, H, 128], BF16, tag="wst", name="wst")
            nc.sync.dma_start(out=t, in_=w_dram[mt].rearrange("p (kt c) -> p kt c", c=128))
            return t

        def proj_T(pool, wkey, bkey, dst_f, nfree, rhs_T):
            """dst_f[:, mt, :] = (W.T @ rhs_T) + bias, for 12 output blocks."""
            for mt in range(H):
                wt = wtiles(pool, W[wkey], mt)
                ps = PS.tile([128, nfree], F32, tag="mm", name="ps_p")
                for kt in range(H):
                    nc.tensor.matmul(ps, lhsT=wt[:, kt, :], rhs=rhs_T[:, kt, :],
                                     start=(kt == 0), stop=(kt == H - 1))
                nc.scalar.activation(out=dst_f[:, mt, :], in_=ps, func=AF.Identity,
                                     bias=BC[bkey][:, mt:mt + 1], scale=1.0)

        def rms_apply(pool, src_f, nkey, dst_bf, nfree, rope):
            """RMS-normalize src_f (over all 1536 dims, per row) * n-weight,
            optional rope, into dst_bf."""
            sq = pool.tile([128, H, nfree], BF16, tag="sq", name="sq")
            for mt in range(H):
                nc.scalar.activation(out=sq[:, mt, :], in_=src_f[:, mt, :], func=AF.Square)
            psd = PS.tile([1, nfree], F32, tag="den", name="ps_rms")
            for mt in range(H):
                nc.tensor.matmul(psd, lhsT=ones_bf, rhs=sq[:, mt, :],
                                 start=(mt == 0), stop=(mt == H - 1))
            rms = pool.tile([1, nfree], F32, tag="rms", name="rms")
            nc.scalar.activation(out=rms, in_=psd, func=AF.Sqrt, bias=eps_t[0:1, :], scale=1.0 / D)
            nc.vector.reciprocal(out=rms, in_=rms)
            rmsb = pool.tile([128, nfree], F32, tag="rmsb", name="rmsb")
            nc.gpsimd.partition_broadcast(rmsb, rms)
            for mt in range(H):
                if not rope:
                    nc.vector.scalar_tensor_tensor(
                        out=dst_bf[:, mt, :], in0=src_f[:, mt, :],
                        scalar=BC[nkey][:, mt:mt + 1], in1=rmsb, op0=ALU.mult, op1=ALU.mult)
                else:
                    sct = pool.tile([128, nfree], BF16, tag="vrow", name="sct")
                    nc.vector.scalar_tensor_tensor(
                        out=sct, in0=src_f[:, mt, :],
                        scalar=BC[nkey][:, mt:mt + 1], in1=rmsb, op0=ALU.mult, op1=ALU.mult)
                    tec = pool.tile([64, nfree], BF16, tag="rta", name="tec")
                    tos = pool.tile([64, nfree], BF16, tag="rtb", name="tos")
                    nc.vector.tensor_tensor(out=tec, in0=sct[0:64, :], in1=ctx["cos"][0:64, :], op=ALU.mult)
                    nc.vector.tensor_tensor(out=tos, in0=sct[64:128, :], in1=ctx["sin"][64:128, :], op=ALU.mult)
                    nc.vector.tensor_tensor(out=dst_bf[0:64, mt, :], in0=tec, in1=tos, op=ALU.subtract)
                    tes = pool.tile([64, nfree], BF16, tag="rta", name="tes")
                    toc = pool.tile([64, nfree], BF16, tag="rtb", name="toc")
                    nc.vector.tensor_tensor(out=tes, in0=sct[0:64, :], in1=ctx["sin"][0:64, :], op=ALU.mult)
                    nc.vector.tensor_tensor(out=toc, in0=sct[64:128, :], in1=ctx["cos"][64:128, :], op=ALU.mult)
                    nc.vector.tensor_tensor(out=dst_bf[64:128, mt, :], in0=tes, in1=toc, op=ALU.add)

        def attention(pool, pool1, q_src, aT_dst, n_kt, kt_of, v_of, masked):
            """Generic attention: q_src [128,H,512] bf16; per head accumulate
            A.T and denominators over n_kt kpos tiles."""
            for h in range(H):
                ps_a = PS.tile([128, SH], F32, tag="at", name="ps_at")
                dacc = pool.tile([128, SH], F32, tag="dacc", name="dacc")
                dacc2 = pool.tile([128, SH], F32, tag="dacc2", name="dacc2")
                exps = [None] * n_kt

                def do_st(t):
                    ps_s = PS.tile([128, SH], F32, tag="mm", name="ps_st")
                    nc.tensor.matmul(ps_s, lhsT=kt_of(pool, h, t), rhs=q_src[:, h, :],
                                     start=True, stop=True)
                    ex = pool.tile([128, SH], BF16, tag="exp", name="exp")
                    if masked and t == n_kt - 1:
                        nc.scalar.activation(out=ex, in_=ps_s, func=AF.Exp, bias=kmask_t, scale=SCALE)
                    else:
                        nc.scalar.activation(out=ex, in_=ps_s, func=AF.Exp, bias=0.0, scale=SCALE)
                    exps[t] = ex

                def do_av(t):
                    nc.tensor.matmul(ps_a, lhsT=v_of(pool, h, t), rhs=exps[t],
                                     start=(t == 0), stop=(t == n_kt - 1))
                    # denominator: two interleaved DVE accumulators (shorter dep chain)
                    if t == 0:
                        nc.vector.tensor_copy(out=dacc, in_=exps[t])
                    elif t == 1:
                        nc.vector.tensor_copy(out=dacc2, in_=exps[t])
                    elif t % 2 == 0:
                        nc.vector.tensor_add(dacc, dacc, exps[t])
                    else:
                        nc.vector.tensor_add(dacc2, dacc2, exps[t])
                    exps[t] = None

                do_st(0)
                if n_kt > 1:
                    do_st(1)
                for t in range(2, n_kt):
                    do_st(t)
                    do_av(t - 2)
                if n_kt > 1:
                    do_av(n_kt - 2)
                do_av(n_kt - 1)
                if n_kt > 1:
                    nc.vector.tensor_add(dacc, dacc, dacc2)
                ps_d = PS.tile([1, SH], F32, tag="den", name="ps_dn")
                nc.tensor.matmul(ps_d, lhsT=ones_f, rhs=dacc, start=True, stop=True)
                inv = pool1.tile([1, SH], F32, tag="inv", name="inv")
                nc.vector.reciprocal(out=inv, in_=ps_d)
                invb = pool1.tile([128, SH], F32, tag="invb", name="invb")
                nc.gpsimd.partition_broadcast(invb, inv)
                nc.vector.tensor_tensor(out=aT_dst[:, h, :], in0=ps_a, in1=invb, op=ALU.mult)

        def oproj_residual_w(pool, wpool, wkey, ob_bc, aT_src, gate):
            # o rows: psum[q, cols] = sum_kt aT[:, kt, qsub].T @ Wo[kt][:, cols]
            for chk in range(3):
                wt = wpool.tile([128, H, SH], BF16, tag="wsto", name="wsto")
                nc.sync.dma_start(
                    out=wt,
                    in_=W[wkey][:, :, chk * SH:(chk + 1) * SH].rearrange("kt p c -> p kt c"))
                sl = slice(chk * SH, (chk + 1) * SH)
                for rt in range(4):
                    ps = PS.tile([128, SH], F32, tag="mm", name="ps_o")
                    for kt in range(H):
                        nc.tensor.matmul(ps, lhsT=aT_src[:, kt, rt * 128:(rt + 1) * 128],
                                         rhs=wt[:, kt, :], start=(kt == 0), stop=(kt == H - 1))
                    u = pool.tile([128, SH], F32, tag="u_o", name="u_o")
                    nc.vector.tensor_tensor(out=u, in0=ps, in1=ob_bc[:, sl], op=ALU.add)
                    if gate is not None:
                        nc.vector.tensor_tensor(out=u, in0=u, in1=gate[:, sl], op=ALU.mult)
                    nc.vector.tensor_tensor(out=x_acc[:, rt, sl], in0=x_acc[:, rt, sl],
                                            in1=u, op=ALU.add)

        # ================= MID scope (sa + ca lifetimes) =================
        with tc.tile_pool(name="MID", bufs=1) as M:
            # --- prologue: processed rows -> DRAM scratch slots ---
            # slots: 0 sc1_msa, 1 sh_msa, 2 g_msa, 3 sh_mlp, 4 sc1_mlp, 5 g_mlp
            def prep_row(scr, tag, row, slot, plus1):
                a = scr.tile([1, D], F32, tag="scrA", name="pa_" + tag)
                nc.sync.dma_start(out=a, in_=io["modul"][row:row + 1, :])
                b = scr.tile([1, D], F32, tag="scrB", name="pb_" + tag)
                nc.sync.dma_start(out=b, in_=io["t_mod"][row:row + 1, :])
                nc.vector.tensor_add(a, a, b)
                if plus1:
                    nc.vector.tensor_scalar_add(a, a, 1.0)
                nc.sync.dma_start(out=dram["scr_rows"][slot:slot + 1, :], in_=a)

            def col_tile(pool, tag, dram_row):
                t = pool.tile([128, H], F32, tag="col_" + tag, name="col_" + tag)
                nc.sync.dma_start(out=t, in_=colv_ap(dram_row))
                return t

            def bc_tile(pool, tag, dram_row):
                t = pool.tile([128, D], BF16, tag="bc_" + tag, name="bc_" + tag)
                tf = pool.tile([128, D], F32, tag="bcf_sh", name="bcf_" + tag)
                nc.sync.dma_start(out=tf, in_=rowbc_ap(dram_row))
                nc.vector.tensor_copy(out=t, in_=tf)
                return t

            with tc.tile_pool(name="PRE", bufs=1) as PRE:
                prep_row(PRE, "sc1_msa", 1, 0, True)
                prep_row(PRE, "sh_msa", 0, 1, False)
                sc1_msa_c = col_tile(M, "sc1_msa", dram["scr_rows"][0:1, :])
                sh_msa_c = col_tile(M, "sh_msa", dram["scr_rows"][1:2, :])

            q_bf = M.tile([128, H, SH], BF16)
            aT = M.tile([128, H, SH], BF16)

            # ---------- Sub1: sa projections + AGs + ca prep ----------
            with tc.tile_pool(name="S1", bufs=1) as S1, tc.tile_pool(name="S1s", bufs=2) as S1s:
                for rt in range(4):
                    ln_rows(S1, rt)
                    rows_to_T1(hT, h_bf, rt, ident_bf, BF16, sc1_msa_c, sh_msa_c)
                prep_row(S1, "g_msa", 2, 2, False)
                prep_row(S1, "sh_mlp", 3, 3, False)
                prep_row(S1, "sc1_mlp", 4, 4, True)
                prep_row(S1, "g_mlp", 5, 5, False)
                g_msa = bc_tile(M, "g_msa", dram["scr_rows"][2:3, :])
                sh_mlp_c = col_tile(G, "sh_mlp", dram["scr_rows"][3:4, :])
                sc1_mlp_c = col_tile(G, "sc1_mlp", dram["scr_rows"][4:5, :])
                g_mlp = bc_tile(G, "g_mlp", dram["scr_rows"][5:6, :])
                sa_ob_b = bc_tile(M, "sa_ob", Rr["sa_ob_r"][:, :])
                ca_ob_b = bc_tile(M, "ca_ob", Rr["ca_ob_r"][:, :])
                n3w_c = col_tile(M, "n3w", Rr["n3w_r"][:, :])
                n3b_c = col_tile(M, "n3b", Rr["n3b_r"][:, :])
                b2_b = bc_tile(G, "b2", Rr["b2_r"][:, :])

                cos_b = S1.tile([128, SH], BF16)
                nc.sync.dma_start(out=cos_b, in_=io["cos_dup"][:, :])
                sin_b = S1.tile([128, SH], BF16)
                nc.sync.dma_start(out=sin_b, in_=io["sin_dup"][:, :])
                ctx["cos"], ctx["sin"] = cos_b, sin_b

                proj_f = S1.tile([128, H, SH], BF16)
                stage_T = S1.tile([128, H, SH], BF16)   # vT / kT staging

                # --- k (rms+rope) + AG ---
                proj_T(S1s, "sa_kw_p", "sa_kb_c", proj_f, SH, hT)
                rms_apply(S1, proj_f, "sa_nk_c", stage_T, SH, rope=True)
                nc.sync.dma_start(out=dram["cc_k_in"][:, :].rearrange("(mt p) c -> p mt c", p=128),
                                  in_=stage_T)
                if SIM_MODE:
                    nc.sync.dma_start(out=dram["cc_k_out"][0], in_=dram["cc_k_in"][:, :])
                else:
                    nc.gpsimd.collective_compute(
                        "AllGather", ALU.bypass, replica_groups=RG,
                        ins=[dram["cc_k_in"][:, :].opt()], outs=[dram["cc_k_out"][:, :, :].opt()])

                # --- v (T orientation, then transpose to rows) + AG ---
                v_T = S1.tile([128, H, SH], BF16, tag="sq", name="v_T")
                proj_T(S1s, "sa_vw", "sa_vb_c", v_T, SH, hT)
                for rt in range(4):
                    vr = T_to_rows(S1, v_T, rt, ident_bf, BF16, "vrow")
                    for hh in range(H):
                        nc.sync.dma_start(out=dram["cc_v_in"][rt, hh],
                                          in_=vr[:, hh * 128:(hh + 1) * 128])
                if SIM_MODE:
                    nc.sync.dma_start(out=dram["cc_v_out"][0], in_=dram["cc_v_in"][:, :, :, :])
                else:
                    nc.gpsimd.collective_compute(
                        "AllGather", ALU.bypass, replica_groups=RG,
                        ins=[dram["cc_v_in"][:, :, :, :].opt()],
                        outs=[dram["cc_v_out"][:, :, :, :, :].opt()])

                # --- q (rms+rope) — its DVE chain overlaps ca-prep PE work ---
                proj_T(S1s, "sa_qw_p", "sa_qb_c", proj_f, SH, hT)
                rms_apply(S1, proj_f, "sa_nq_c", q_bf, SH, rope=True)

                # --- ca context prep (fills the AG + q-rope window) ---
                nc.sync.dma_start(out=h_bf, in_=io["ctx_bf"][:, :].rearrange("(rt p) c -> p rt c", p=128))
                ctxT = S1.tile([128, H, LC], BF16)
                rows_to_T(ctxT, h_bf, ident_bf, BF16)
                # ca v
                proj_T(S1s, "ca_vw", "ca_vb_c", stage_T, LC, ctxT)
                for rt in range(4):
                    vr = T_to_rows(S1, stage_T, rt, ident_bf, BF16, "vrow")
                    nc.sync.dma_start(out=dram["ca_v_dram"][rt * 128:(rt + 1) * 128, :], in_=vr)
                # ca kT
                proj_T(S1s, "ca_kw", "ca_kb_c", proj_f, LC, ctxT)
                rms_apply(S1, proj_f, "ca_nk_c", stage_T, LC, rope=False)
                nc.sync.dma_start(out=dram["ca_k_dram"][:, :].rearrange("(mt p) c -> p mt c", p=128),
                                  in_=stage_T)

            # ---------- Sub2: self-attention + o-proj + residual ----------
            with tc.tile_pool(name="S2", bufs=1) as S2, tc.tile_pool(name="S2s", bufs=3) as S2s:
                def sa_kt(pool, h, t):
                    b, r = t // 4, t % 4
                    if r == 0:
                        nt = 4 if b < 7 else 1
                        kt_b = pool.tile([128, 4, 128], BF16, tag="kt", name="kt_b")
                        nc.sync.dma_start(out=kt_b[:, 0:nt, :].rearrange("p n c -> p (n c)"),
                                          in_=dram["cc_k_out"][b, h * 128:(h + 1) * 128, 0:nt * 128])
                        vh_b = pool.tile([128, 4, 128], BF16, tag="vh", name="vh_b")
                        for rb in range(nt):
                            nc.sync.dma_start(out=vh_b[:, rb, :], in_=dram["cc_v_out"][b, rb, h])
                        ctx["kt_b"], ctx["vh_b"] = kt_b, vh_b
                    return ctx["kt_b"][:, r, :]

                def sa_v(pool, h, t):
                    return ctx["vh_b"][:, t % 4, :]

                attention(S2s, S2, q_bf, aT, NKT, sa_kt, sa_v, masked=True)
                oproj_residual_w(S2, S2s, "sa_ow", sa_ob_b, aT, g_msa)

            # ---------- Sub3: cross-attention ----------
            with (tc.tile_pool(name="S3", bufs=1) as S3,
                  tc.tile_pool(name="S3s", bufs=3) as S3s,
                  tc.tile_pool(name="S3w", bufs=2) as S3w):
                for rt in range(4):
                    ln_rows(S3, rt)
                    rows_to_T1(hT, h_bf, rt, ident_bf, BF16, n3w_c, n3b_c)
                proj_f = S3.tile([128, H, SH], BF16, tag="oT")   # shares slot with oT
                proj_T(S3w, "ca_qw", "ca_qb_c", proj_f, SH, hT)
                rms_apply(S3, proj_f, "ca_nq_c", q_bf, SH, rope=False)

                def ca_kt(pool, h, t):
                    if t == 0:
                        kt_b = pool.tile([128, LC], BF16, tag="kt", name="kt_c")
                        nc.sync.dma_start(out=kt_b, in_=dram["ca_k_dram"][h * 128:(h + 1) * 128, :])
                        vh_b = pool.tile([128, 4, 128], BF16, tag="vh", name="vh_c")
                        nc.sync.dma_start(
                            out=vh_b,
                            in_=dram["ca_v_dram"][:, :].rearrange("(rb p) c -> p rb c", p=128)[:, :, h * 128:(h + 1) * 128])
                        ctx["kt_b"], ctx["vh_b"] = kt_b, vh_b
                    return ctx["kt_b"][:, t * 128:(t + 1) * 128]

                def ca_v(pool, h, t):
                    return ctx["vh_b"][:, t, :]

                attention(S3s, S3, q_bf, aT, 4, ca_kt, ca_v, masked=False)
                oproj_residual_w(S3, S3w, "ca_ow", ca_ob_b, aT, None)

                # FFN input: LN + modulate + transpose + AG
                for rt in range(4):
                    ln_rows(S3, rt)
                    rows_to_T1(hT, h_bf, rt, ident_bf, BF16, sc1_mlp_c, sh_mlp_c)
                nc.sync.dma_start(out=dram["cc_h_in"][:, :].rearrange("(mt p) c -> p mt c", p=128),
                                  in_=hT)
                if SIM_MODE:
                    nc.sync.dma_start(out=dram["cc_h_out"][0], in_=dram["cc_h_in"][:, :])
                else:
                    nc.gpsimd.collective_compute(
                        "AllGather", ALU.bypass, replica_groups=RG,
                        ins=[dram["cc_h_in"][:, :].opt()], outs=[dram["cc_h_out"][:, :, :].opt()])

        # ================= FFN =================
        with tc.tile_pool(name="FF", bufs=1) as FF, tc.tile_pool(name="FFs", bufs=2) as FFs:
            w1_sb = FF.tile([128, 9, H, 128], BF16)
            nc.sync.dma_start(out=w1_sb, in_=W["w1_s"][:, :].rearrange(
                "p (m kt c) -> p m kt c", m=9, kt=H))

            pid = nc.sync.partition_id()
            for i in range(8):
                # chunk (pid + i) % 8: local chunk first so mm1 starts before AG(h) lands
                idx = (pid + i) % 8
                if i == 0:
                    rhs_T = hT
                else:
                    hTc = FFs.tile([128, H, SH], BF16, tag="hTc", name="hTc")
                    srcb = dram["cc_h_out"][bass.ds(idx, 1), :, :]
                    nc.sync.dma_start(
                        out=hTc, in_=srcb.rearrange("o (dt p) c -> (o p) dt c", p=128))
                    rhs_T = hTc
                y1 = FF.tile([128, 9, SH], BF16, tag="y1", name="y1")
                for m in range(9):
                    ps = PS.tile([128, SH], F32, tag="mm", name="ps_f1")
                    for kt in range(H):
                        nc.tensor.matmul(ps, lhsT=w1_sb[:, m, kt, :], rhs=rhs_T[:, kt, :],
                                         start=(kt == 0), stop=(kt == H - 1))
                    nc.scalar.activation(out=y1[:, m, :], in_=ps, func=AF.Gelu_apprx_tanh,
                                         bias=BC["b1_c"][:, m:m + 1], scale=1.0)
                yc = FF.tile([128, H, SH], BF16, tag="yc", name="yc")
                for m2 in range(H):
                    w2t = FFs.tile([128, 9, 128], BF16, tag="w2st", name="w2t")
                    nc.sync.dma_start(out=w2t, in_=W["w2_s"][:, m2 * 9 * 128:(m2 + 1) * 9 * 128]
                                      .rearrange("p (kt c) -> p kt c", c=128))
                    ps = PS.tile([128, SH], F32, tag="at", name="ps_f2")
                    for k2 in range(9):
                        nc.tensor.matmul(ps, lhsT=w2t[:, k2, :], rhs=y1[:, k2, :],
                                         start=(k2 == 0), stop=(k2 == 8))
                    nc.vector.tensor_copy(out=yc[:, m2, :], in_=ps)
                dstb = dram["cc_y_in"][bass.ds(idx, 1), :, :]
                nc.sync.dma_start(out=dstb.rearrange("o (mt p) c -> (o p) mt c", p=128),
                                  in_=yc)

            if SIM_MODE:
                nc.sync.dma_start(out=dram["cc_y_out"][:, :], in_=dram["cc_y_in"][0])
            else:
                nc.gpsimd.collective_compute(
                    "ReduceScatter", ALU.add, replica_groups=RG,
                    ins=[dram["cc_y_in"][:, :, :].opt()], outs=[dram["cc_y_out"][:, :].opt()])

            y2T = FF.tile([128, H, SH], BF16)
            nc.sync.dma_start(out=y2T, in_=dram["cc_y_out"][:, :].rearrange("(dt p) c -> p dt c", p=128))
            for rt in range(4):
                yrow = T_to_rows(FF, y2T, rt, ident_bf, BF16, "yrow")
                t1 = FF.tile([128, D], F32, tag="t1", name="t1")
                nc.vector.tensor_tensor(out=t1, in0=yrow, in1=b2_b, op=ALU.add)
                t2 = FF.tile([128, D], F32, tag="t2", name="t2")
                nc.vector.tensor_tensor(out=t2, in0=t1, in1=g_mlp, op=ALU.mult)
                nc.vector.tensor_tensor(out=t1, in0=t2, in1=x_acc[:, rt, :], op=ALU.add)
                nc.sync.dma_start(out=y_out[rt * 128:(rt + 1) * 128, :], in_=t1)


# ---------------- host side ----------------
_NC_CACHE = None


def _get_nc():
    global _NC_CACHE
    if _NC_CACHE is None:
        _NC_CACHE = build()
    return _NC_CACHE


def _prep(inputs):
    f32 = np.float32
    perm_head = np.concatenate([np.arange(0, 128, 2), np.arange(1, 128, 2)])
    full_perm = np.concatenate([128 * h + perm_head for h in range(H)])

    x = np.asarray(inputs["x"], f32).reshape(S, D)
    x_pad = np.zeros((SP, D), f32)
    x_pad[:S] = x
    ctx_b = np.asarray(inputs["context"], f32).reshape(LC, D).astype(BF)
    modul = np.asarray(inputs["modulation"], f32).reshape(6, D)
    t_mod = np.asarray(inputs["t_mod"], f32).reshape(6, D)

    cos = np.asarray(inputs["rope_cos"], f32)
    sin = np.asarray(inputs["rope_sin"], f32)
    cos_pad = np.ones((SP, 64), f32)
    sin_pad = np.zeros((SP, 64), f32)
    cos_pad[:S] = cos
    sin_pad[:S] = sin

    kmask = np.where(np.arange(128) < 16, 0.0, NEG).astype(f32).reshape(128, 1)

    def colmat(v, perm=None):
        v = np.asarray(v, f32).reshape(D)
        if perm is not None:
            v = v[perm]
        return np.ascontiguousarray(v.reshape(H, 128).T)

    def wtile(w):
        # [1536,1536] -> [mt, p, kt, c] with tile[mt, p, kt*128+c] = W[kt*128+p, mt*128+c]
        w = np.asarray(w, f32).reshape(H, 128, H, 128)
        return np.ascontiguousarray(w.transpose(2, 1, 0, 3).reshape(H, 128, D)).astype(BF)

    shared = dict(
        ctx_bf=ctx_b, modul=modul, t_mod=t_mod, kmask=kmask,
        sa_qw_p=wtile(np.asarray(inputs["sa_qw"], f32)[:, full_perm]),
        sa_kw_p=wtile(np.asarray(inputs["sa_kw"], f32)[:, full_perm]),
        sa_vw=wtile(inputs["sa_vw"]),
        sa_ow=np.asarray(inputs["sa_ow"], f32).reshape(H, 128, D).astype(BF),
        ca_qw=wtile(inputs["ca_qw"]),
        ca_kw=wtile(inputs["ca_kw"]),
        ca_vw=wtile(inputs["ca_vw"]),
        ca_ow=np.asarray(inputs["ca_ow"], f32).reshape(H, 128, D).astype(BF),
        sa_qb_c=colmat(inputs["sa_qb"], full_perm),
        sa_kb_c=colmat(inputs["sa_kb"], full_perm),
        sa_nq_c=colmat(inputs["sa_nq"], full_perm),
        sa_nk_c=colmat(inputs["sa_nk"], full_perm),
        sa_vb_c=colmat(inputs["sa_vb"]),
        ca_qb_c=colmat(inputs["ca_qb"]),
        ca_kb_c=colmat(inputs["ca_kb"]),
        ca_nq_c=colmat(inputs["ca_nq"]),
        ca_nk_c=colmat(inputs["ca_nk"]),
        ca_vb_c=colmat(inputs["ca_vb"]),
        sa_ob_r=np.asarray(inputs["sa_ob"], f32).reshape(1, D),
        ca_ob_r=np.asarray(inputs["ca_ob"], f32).reshape(1, D),
        n3w_r=np.asarray(inputs["n3_w"], f32).reshape(1, D),
        n3b_r=np.asarray(inputs["n3_b"], f32).reshape(1, D),
        b2_r=np.asarray(inputs["ffn_b2"], f32).reshape(1, D),
    )

    w1 = np.asarray(inputs["ffn_w1"], f32)
    w2 = np.asarray(inputs["ffn_w2"], f32)
    b1 = np.asarray(inputs["ffn_b1"], f32)

    in_maps = []
    for c in range(N_CORES):
        w1s = np.zeros((D, FSHP), f32)
        w1s[:, :FSH] = w1[:, c * FSH:(c + 1) * FSH]
        w2s = np.zeros((FSHP, D), f32)
        w2s[:FSH] = w2[c * FSH:(c + 1) * FSH]
        b1s = np.zeros(FSHP, f32)
        b1s[:FSH] = b1[c * FSH:(c + 1) * FSH]
        ct = cos_pad[c * SH:(c + 1) * SH].T
        st = sin_pad[c * SH:(c + 1) * SH].T
        m = dict(shared)
        m.update(
            x_sh=np.ascontiguousarray(x_pad[c * SH:(c + 1) * SH]),
            cos_dup=np.ascontiguousarray(np.concatenate([ct, ct], axis=0)).astype(BF),
            sin_dup=np.ascontiguousarray(np.concatenate([st, st], axis=0)).astype(BF),
            w1_s=np.ascontiguousarray(
                w1s.reshape(H, 128, 9, 128).transpose(1, 2, 0, 3).reshape(128, 9 * H * 128)).astype(BF),
            w2_s=np.ascontiguousarray(
                w2s.reshape(9, 128, H, 128).transpose(1, 2, 0, 3).reshape(128, H * 9 * 128)).astype(BF),
            b1_c=np.ascontiguousarray(b1s.reshape(9, 128).T),
        )
        in_maps.append(m)
    return in_maps


def kernel(**inputs):
    nc = _get_nc()
    in_maps = _prep(inputs)
    res = run_bass_kernel_spmd(nc, in_maps, core_ids=list(range(N_CORES)))
    out = np.concatenate([res.results[c]["y_out"] for c in range(N_CORES)], axis=0)[:S]
    return out.reshape(1, S, D).astype(np.float32)
